# revision 10
# baseline (speedup 1.0000x reference)
"""AASIST backbone (conv stack + 2 GAT layers + attention pool) on 8 TRN2 NeuronCores.

Strategy: data-parallel over batch. B=4 -> batch element b runs on cores {b, b+4}
(duplicated; host reads cores 0-3). Whole network for one batch element runs on
one core:

  - Conv1d stack (7 convs, K=3, BN folded into weights on host) as 3 PSUM-
    accumulating matmuls per conv in [C, T] layout (channels on partitions).
  - GAT layers in transposed layout h^T = [feat, node]: scores e^T[j, n] =
    f1[n] + f2[j] built in PSUM via a K=1 broadcast matmul + per-partition ACT
    bias; Prelu(alpha=0.2) + Exp on ACT; aggregation + softmax denominator via
    one [128, 65] matmul per j-chunk (ones column = Z row); 1/Z via exp(-ln Z).
  - Attention pool with softmax numerator/denominator; final division on host.

All matmuls run in float32r (full-rate fp32 on the PE).
"""

import numpy as np

import concourse.bacc as bacc
import concourse.bass as bass
import concourse.tile as tile
import concourse.mybir as mybir
from concourse.bass_utils import run_bass_kernel_spmd

F32 = mybir.dt.float32
F32R = mybir.dt.float32r
AF = mybir.ActivationFunctionType
ALU = mybir.AluOpType

N_CORES = 8
B = 4
T = 512
TP = 516   # padded activation width; data lives in cols [2, 514)
D0 = 2
BN_EPS = 1e-5
ALPHA = 0.2

# ---------------------------------------------------------------- blob layout
# wA [128, CA] f32r : 128-row matmul weights
A_RB3C2 = 0          # [128, 3, 128] -> 384
A_W1GAT = 384        # [128, 64]
A_W1A = 448          # [128, 2]
CA = 450
# wB [64, CB] f32r : 64-row matmul weights + row-0 oddballs
B_RB1W1 = 0          # [64, 3, 64]
B_RB1W2 = 192
B_RB2W1 = 384
B_RB2W2 = 576
B_RB3C1 = 768        # [64, 3, 128] -> 384
B_SKW = 1152         # [64, 128]
B_I64 = 1280         # [64, 64]
B_PW1 = 1344         # [64, 32]
B_W2GAT = 1376       # [64, 64]
B_W2A = 1440         # [64, 2]
B_CT = 1442          # row 0 only: [1, 3, 64] -> 192
B_ONES = 1634        # row 0 only: [1, 512] of 1.0
B_PW2 = 2146         # [32, 1]
CB = 2148
# wC [128, CC] f32 : ACT bias columns
C_B0, C_B11, C_B12, C_B21, C_B22, C_B31, C_B32, C_PB1 = range(8)
CC = 8


def _fold_bn(w, b, bnp):
    g, beta, m, v = bnp
    s = g / np.sqrt(v + BN_EPS)
    return w * s[:, None, None], (b - m) * s + beta


def _lhsT(w):
    # conv weight (O, I, K) -> lhsT layout [I, K, O]
    return np.transpose(w, (1, 2, 0))


def _prep(x, ct, resblocks, gats, pool):
    """Host-side: fold BN, pack weight blobs, return (wA, wB, wC, xs)."""
    f = lambda a: np.asarray(a, dtype=np.float32)

    wA = np.zeros((128, CA), np.float32)
    wB = np.zeros((64, CB), np.float32)
    wC = np.zeros((128, CC), np.float32)

    ctw, ctb = f(ct[0]), f(ct[1])
    r1, r2, r3 = resblocks
    w11, b11 = _fold_bn(f(r1["c1w"]), f(r1["c1b"]), [f(p) for p in r1["bn1"]])
    w12, b12 = _fold_bn(f(r1["c2w"]), f(r1["c2b"]), [f(p) for p in r1["bn2"]])
    w21, b21 = _fold_bn(f(r2["c1w"]), f(r2["c1b"]), [f(p) for p in r2["bn1"]])
    w22, b22 = _fold_bn(f(r2["c2w"]), f(r2["c2b"]), [f(p) for p in r2["bn2"]])
    w31, b31 = _fold_bn(f(r3["c1w"]), f(r3["c1b"]), [f(p) for p in r3["bn1"]])
    w32, b32 = _fold_bn(f(r3["c2w"]), f(r3["c2b"]), [f(p) for p in r3["bn2"]])
    skw, skb = f(r3["skw"]), f(r3["skb"])

    wA[:, A_RB3C2:A_RB3C2 + 384] = _lhsT(w32).reshape(128, 384)
    W1, a1 = f(gats[0][0]), f(gats[0][1])
    wA[:, A_W1GAT:A_W1GAT + 64] = W1
    wA[:, A_W1A:A_W1A + 2] = W1 @ np.stack([a1[:64, 0], a1[64:, 0]], axis=1)

    wB[:, B_RB1W1:B_RB1W1 + 192] = _lhsT(w11).reshape(64, 192)
    wB[:, B_RB1W2:B_RB1W2 + 192] = _lhsT(w12).reshape(64, 192)
    wB[:, B_RB2W1:B_RB2W1 + 192] = _lhsT(w21).reshape(64, 192)
    wB[:, B_RB2W2:B_RB2W2 + 192] = _lhsT(w22).reshape(64, 192)
    wB[:, B_RB3C1:B_RB3C1 + 384] = _lhsT(w31).reshape(64, 384)
    wB[:, B_SKW:B_SKW + 128] = skw[:, :, 0].T
    wB[:, B_I64:B_I64 + 64] = np.eye(64, dtype=np.float32)
    pw1, pb1, pw2, _pb2 = [f(p) for p in pool]  # pb2 shifts all logits: dropped
    wB[:, B_PW1:B_PW1 + 32] = pw1
    W2, a2 = f(gats[1][0]), f(gats[1][1])
    wB[:, B_W2GAT:B_W2GAT + 64] = W2
    wB[:, B_W2A:B_W2A + 2] = W2 @ np.stack([a2[:64, 0], a2[64:, 0]], axis=1)
    wB[0, B_CT:B_CT + 192] = _lhsT(ctw).reshape(1, 192)
    wB[0, B_ONES:B_ONES + 512] = 1.0
    wB[:32, B_PW2] = pw2[:, 0]

    wC[:64, C_B0] = ctb
    wC[:64, C_B11] = b11
    wC[:64, C_B12] = b12
    wC[:64, C_B21] = b21
    wC[:64, C_B22] = b22
    wC[:, C_B31] = b31
    wC[:, C_B32] = b32 + skb
    wC[:32, C_PB1] = pb1

    xs = np.zeros((B, TP), np.float32)
    xs[:, D0:D0 + T] = f(x)[:, 0, :]
    return wA, wB, wC, xs


# ------------------------------------------------------------------ device IR

def _emit(ctx, tc, nc, wA_e, wB_e, wC_e, x_e, out_e):
    wts = ctx.enter_context(tc.tile_pool(name="wts", bufs=1))
    acts = ctx.enter_context(tc.tile_pool(name="acts", bufs=1))
    gp = ctx.enter_context(tc.tile_pool(name="gp", bufs=2))
    ps_conv = ctx.enter_context(tc.tile_pool(name="ps_conv", bufs=2, space="PSUM"))
    ps_big = ctx.enter_context(tc.tile_pool(name="ps_big", bufs=1, space="PSUM"))
    ps_sm = ctx.enter_context(tc.tile_pool(name="ps_sm", bufs=3, space="PSUM"))

    wA = wts.tile([128, CA], F32R)
    wB = wts.tile([64, CB], F32R)
    wC = wts.tile([128, CC], F32)
    x_sb = wts.tile([1, TP], F32R)
    onescol = wts.tile([128, 1], F32)
    nc.sync.dma_start(out=wB[:], in_=wB_e.ap())
    nc.sync.dma_start(out=wA[:], in_=wA_e.ap())
    nc.sync.dma_start(out=wC[:], in_=wC_e.ap())
    nc.sync.dma_start(out=x_sb[:], in_=x_e.ap())
    nc.vector.memset(onescol[:], 1.0)

    ones = wB[0:1, B_ONES:B_ONES + 512]  # [1, 512] of 1.0, f32r

    def mm(out, lhsT, rhs, start, stop):
        nc.tensor.matmul(out=out, lhsT=lhsT, rhs=rhs, start=start, stop=stop,
                         skip_group_check=True)

    def conv3(psum, h_in, w3, extra=None):
        """psum[:, t] = sum_k w3[:, k, :].T @ h_in[:, D0+t+k-1] (+ extra mms).

        h_in is a padded [C, TP] tile (zeros in cols 0:2 and 514:516), so all
        three taps write the full aligned psum [C, 0:T] (fp32r dst rule).
        """
        for k in range(3):
            mm(psum[:, :], w3[:, k, :], h_in[:, k + 1:k + 1 + T],
               k == 0, k == 2 and extra is None)
        if extra is not None:
            lhsT, rhs = extra
            mm(psum[:, :], lhsT, rhs, False, True)

    def padtile(pool_, c, **kw):
        t = pool_.tile([c, TP], F32R, **kw)
        nc.gpsimd.memset(t[:, 0:D0].bitcast(F32), 0.0)
        nc.gpsimd.memset(t[:, D0 + T:TP].bitcast(F32), 0.0)
        return t

    def epilogue(dst, psum, bias_col, func):
        nc.scalar.activation(out=dst[:, D0:D0 + T], in_=psum[:],
                             bias=wC[0:psum.shape[0], bias_col:bias_col + 1],
                             func=func, scale=1.0)

    # ---------------- conv stack ----------------
    with nc.named_scope("convs"):
        ctw = wB[0:1, B_CT:B_CT + 192].rearrange("p (k o) -> p k o", k=3)
        h0 = padtile(acts, 64, tag="h0")
        p = ps_conv.tile([64, T], F32, tag="cps")
        conv3(p, x_sb, ctw)
        epilogue(h0, p, C_B0, AF.Identity)

        rb1w1 = wB[:, B_RB1W1:B_RB1W1 + 192].rearrange("p (k o) -> p k o", k=3)
        s1 = padtile(acts, 64, tag="s1")
        p = ps_conv.tile([64, T], F32, tag="cps")
        conv3(p, h0, rb1w1)
        epilogue(s1, p, C_B11, AF.Relu)

        rb1w2 = wB[:, B_RB1W2:B_RB1W2 + 192].rearrange("p (k o) -> p k o", k=3)
        h1 = padtile(acts, 64, tag="h1")
        p = ps_conv.tile([64, T], F32, tag="cps")
        conv3(p, s1, rb1w2, extra=(wB[:, B_I64:B_I64 + 64], h0[:, D0:D0 + T]))
        epilogue(h1, p, C_B12, AF.Relu)

        rb2w1 = wB[:, B_RB2W1:B_RB2W1 + 192].rearrange("p (k o) -> p k o", k=3)
        s2 = padtile(acts, 64, tag="s2")
        p = ps_conv.tile([64, T], F32, tag="cps")
        conv3(p, h1, rb2w1)
        epilogue(s2, p, C_B21, AF.Relu)

        rb2w2 = wB[:, B_RB2W2:B_RB2W2 + 192].rearrange("p (k o) -> p k o", k=3)
        h2 = padtile(acts, 64, tag="h2")
        p = ps_conv.tile([64, T], F32, tag="cps")
        conv3(p, s2, rb2w2, extra=(wB[:, B_I64:B_I64 + 64], h1[:, D0:D0 + T]))
        epilogue(h2, p, C_B22, AF.Relu)

        rb3c1 = wB[:, B_RB3C1:B_RB3C1 + 384].rearrange("p (k o) -> p k o", k=3)
        s3 = padtile(acts, 128, tag="s3")
        p = ps_conv.tile([128, T], F32, tag="cps")
        conv3(p, h2, rb3c1)
        epilogue(s3, p, C_B31, AF.Relu)

        rb3c2 = wA[:, A_RB3C2:A_RB3C2 + 384].rearrange("p (k o) -> p k o", k=3)
        h3 = padtile(acts, 128, tag="h3")
        p = ps_conv.tile([128, T], F32, tag="cps")
        conv3(p, s3, rb3c2, extra=(wB[:, B_SKW:B_SKW + 128], h2[:, D0:D0 + T]))
        epilogue(h3, p, C_B32, AF.Relu)

    # ---------------- GAT layers ----------------
    def gat(tag, hT, C, W_sb, Wa_sb):
        with nc.named_scope(tag):
            wh = []
            for jc in range(4):
                wp = ps_sm.tile([128, 64], F32, tag="sm")
                mm(wp[:], hT[:, jc * 128:(jc + 1) * 128], W_sb, True, True)
                ws = gp.tile([128, 65], F32R, tag="whsb")
                nc.vector.tensor_copy(ws[:, 0:64], wp[:])
                nc.vector.tensor_copy(ws[:, 64:65], onescol[:])
                wh.append(ws)

            f1p = ps_sm.tile([1, T], F32, tag="sm")
            mm(f1p[:], Wa_sb[:, 0:1], hT[:, :], True, True)
            f1row = gp.tile([1, T], F32R, tag="f1row")
            nc.vector.tensor_copy(f1row[:], f1p[:])

            f2c = gp.tile([128, 4], F32, tag="f2c")
            for jc in range(4):
                f2p = ps_sm.tile([128, 2], F32, tag="sm")
                mm(f2p[:], hT[:, jc * 128:(jc + 1) * 128], Wa_sb[:, 0:2], True, True)
                nc.vector.tensor_copy(f2c[:, jc:jc + 1], f2p[:, 1:2])

            f1b = ps_big.tile([128, T], F32, tag="f1b")
            mm(f1b[:], ones[:, 0:128], f1row[:], True, True)

            lre = gp.tile([128, 4, T], F32R, tag="lre")
            for jc in range(4):
                nc.scalar.activation(out=lre[:, jc, :], in_=f1b[:],
                                     func=AF.Prelu, alpha=ALPHA,
                                     bias=f2c[:, jc:jc + 1], scale=1.0)
            expE = gp.tile([128, 4, T], F32R, tag="expE")
            nc.scalar.activation(out=expE[:].rearrange("p a b -> p (a b)"),
                                 in_=lre[:].rearrange("p a b -> p (a b)"),
                                 func=AF.Exp)

            o_ps = ps_big.tile([65, T], F32, tag="ops")
            for jc in range(4):
                mm(o_ps[:], wh[jc][:], expE[:, jc, :], jc == 0, jc == 3)

            rln = gp.tile([1, T], F32R, tag="rln")
            nc.scalar.activation(out=rln[:], in_=o_ps[64:65, :], func=AF.Ln)
            rlnb = ps_sm.tile([64, T], F32, tag="sm")
            mm(rlnb[:], ones[:, 0:64], rln[:], True, True)
            rzb = gp.tile([64, T], F32, tag="rzb")
            nc.scalar.activation(out=rzb[:], in_=rlnb[:], func=AF.Exp, scale=-1.0)

            q = gp.tile([64, T], F32, tag="q")
            nc.vector.tensor_tensor(out=q[:], in0=o_ps[0:64, :], in1=rzb[:],
                                    op=ALU.mult)
            e1 = gp.tile([64, T], F32, tag="e1")
            nc.scalar.activation(out=e1[:], in_=q[:], func=AF.Exp)
            m = gp.tile([64, T], F32, tag="m")
            nc.vector.tensor_scalar(out=m[:], in0=e1[:], scalar1=1.0, scalar2=0.0,
                                    op0=ALU.subtract, op1=ALU.min)
            hn = gp.tile([64, T], F32R, tag="hn")
            nc.vector.scalar_tensor_tensor(out=hn[:], in0=q[:], scalar=0.0,
                                           in1=m[:], op0=ALU.max, op1=ALU.add)
            return hn

    h1p = gat("gat1", h3[:, D0:D0 + T], 128, wA[:, A_W1GAT:A_W1GAT + 64], wA[:, A_W1A:A_W1A + 2])
    h2p = gat("gat2", h1p, 64, wB[:, B_W2GAT:B_W2GAT + 64], wB[:, B_W2A:B_W2A + 2])

    # ---------------- attention pool ----------------
    with nc.named_scope("pool"):
        t1p = ps_sm.tile([32, T], F32, tag="sm")
        mm(t1p[:], wB[:, B_PW1:B_PW1 + 32], h2p[:], True, True)
        t1 = gp.tile([32, T], F32R, tag="t1")
        nc.scalar.activation(out=t1[:], in_=t1p[:], func=AF.Relu,
                             bias=wC[0:32, C_PB1:C_PB1 + 1], scale=1.0)

        lp = ps_sm.tile([1, T], F32, tag="sm")
        mm(lp[:], wB[0:32, B_PW2:B_PW2 + 1], t1[:], True, True)
        expl = gp.tile([1, T], F32R, tag="expl")
        z2 = gp.tile([1, 1], F32, tag="z2")
        nc.scalar.activation(out=expl[:], in_=lp[:], func=AF.Exp, accum_out=z2[:])

        eb = ps_sm.tile([64, T], F32, tag="sm")
        mm(eb[:], ones[:, 0:64], expl[:], True, True)
        w_sb = gp.tile([64, T], F32, tag="w_sb")
        oun = gp.tile([64, 1], F32, tag="oun")
        nc.vector.tensor_tensor(out=w_sb[:], in0=h2p[:].bitcast(F32), in1=eb[:],
                                op=ALU.mult)
        nc.vector.reduce_sum(out=oun[:], in_=w_sb[:], axis=mybir.AxisListType.X)
        nc.sync.dma_start(out=out_e.ap()[0:64, :], in_=oun[:])
        nc.sync.dma_start(out=out_e.ap()[64:65, :], in_=z2[:])


_GRAPH = None


def _build():
    global _GRAPH
    if _GRAPH is not None:
        return _GRAPH
    nc = bacc.Bacc("TRN2", target_bir_lowering=False, debug=False,
                   num_devices=N_CORES)
    wA_e = nc.dram_tensor("wA", [128, CA], F32R, kind="ExternalInput")
    wB_e = nc.dram_tensor("wB", [64, CB], F32R, kind="ExternalInput")
    wC_e = nc.dram_tensor("wC", [128, CC], F32, kind="ExternalInput")
    x_e = nc.dram_tensor("x", [1, TP], F32R, kind="ExternalInput")
    out_e = nc.dram_tensor("out", [65, 1], F32, kind="ExternalOutput")
    from contextlib import ExitStack
    with tile.TileContext(nc) as tc:
        with ExitStack() as ctx:
            _emit(ctx, tc, nc, wA_e, wB_e, wC_e, x_e, out_e)
    nc.compile()
    _GRAPH = nc
    return nc


def kernel(x, ct, resblocks, gats, pool, _want_results=False, **_ignored):
    wA, wB, wC, xs = _prep(x, ct, resblocks, gats, pool)
    nc = _build()
    in_maps = [
        {"wA": wA, "wB": wB, "wC": wC, "x": xs[i % B].reshape(1, TP).copy()}
        for i in range(N_CORES)
    ]
    res = run_bass_kernel_spmd(nc, in_maps, core_ids=list(range(N_CORES)))
    out = np.empty((B, 64), np.float32)
    for b in range(B):
        v = res.results[b]["out"][:, 0]
        out[b] = v[0:64] / v[64]
    if _want_results:
        return out, res
    return out


# revision 11
# speedup vs baseline: 1.1137x; 1.1137x over previous
"""AASIST backbone (conv stack + 2 GAT layers + attention pool) on 8 TRN2 NeuronCores.

Strategy: data-parallel over batch. B=4 -> batch element b runs on cores {b, b+4}
(duplicated; host reads cores 0-3). Whole network for one batch element runs on
one core:

  - Conv1d stack (7 convs, K=3, BN folded into weights on host) as 3 PSUM-
    accumulating matmuls per conv in [C, T] layout (channels on partitions);
    resblock identity/skip adds are extra matmuls into the same PSUM group.
    Epilogue (bias+relu) split across ScalarE (first half) and VectorE
    (second half) so the two halves run concurrently.
  - GAT layers in transposed layout h^T = [feat, node]: score matrix
    e^T[j, n] = f1[n] + f2[j]; f1 broadcast comes from one matmul with a
    host-replicated [C, 128] weight; f2[j] enters as the per-partition ACT
    bias; Prelu(alpha=0.2) + Exp per j-chunk on ScalarE (all ACT functions
    stay inside the `exp_and_others` table set: one table load per kernel);
    aggregation + softmax denominator via one [128, 65] matmul per chunk
    (ones column = row of Z); 1/Z via DVE reciprocal_approx_fast on the
    matmul-broadcast Z.
  - Attention pool with softmax numerator/denominator; division on host.

All matmuls run in float32r (full-rate fp32 on the PE; even-N PSUM writes).
"""

import numpy as np

import concourse.bacc as bacc
import concourse.bass as bass
import concourse.tile as tile
import concourse.mybir as mybir
from concourse.bass_utils import run_bass_kernel_spmd

F32 = mybir.dt.float32
F32R = mybir.dt.float32r
AF = mybir.ActivationFunctionType
ALU = mybir.AluOpType

N_CORES = 8
B = 4
T = 512
TP = 516   # padded activation width; data lives in cols [2, 514)
D0 = 2
BN_EPS = 1e-5
ALPHA = 0.2

# ---------------------------------------------------------------- blob layout
# wA [128, CA] f32r : 128-row matmul weights
A_RB3C2 = 0          # [128, 3, 128] -> 384
A_W1GAT = 384        # [128, 64]
A_W1A = 448          # [128, 2]
A_W1AB = 450         # [128, 128]  a1.W1 replicated to 128 cols
CA = 578
# wB [64, CB] f32r : 64-row matmul weights
B_RB1W1 = 0          # [64, 3, 64]
B_RB1W2 = 192
B_RB2W1 = 384
B_RB2W2 = 576
B_RB3C1 = 768        # [64, 3, 128] -> 384
B_SKW = 1152         # [64, 128]
B_I64 = 1280         # [64, 64]
B_PW1 = 1344         # [64, 32]
B_W2GAT = 1376       # [64, 64]
B_W2A = 1440         # [64, 2]
B_PW2 = 1442         # [32, 1]
B_W2AB = 1444        # [64, 128]  a1.W2 replicated
CB = 1572
# wC [128, CC] f32 : ACT bias columns
C_B0, C_B11, C_B12, C_B21, C_B22, C_B31, C_B32, C_PB1 = range(8)
CC = 8
# wD [1, CD] f32r : single-row operands
D_CT = 0             # [1, 3, 64]
D_ONES = 192         # [1, 512] of 1.0
CD = 704


def _fold_bn(w, b, bnp):
    g, beta, m, v = bnp
    s = g / np.sqrt(v + BN_EPS)
    return w * s[:, None, None], (b - m) * s + beta


def _lhsT(w):
    # conv weight (O, I, K) -> lhsT layout [I, K, O]
    return np.transpose(w, (1, 2, 0))


def _prep(x, ct, resblocks, gats, pool):
    """Host-side: fold BN, pack weight blobs, return (wA, wB, wC, wD, xs)."""
    f = lambda a: np.asarray(a, dtype=np.float32)

    wA = np.zeros((128, CA), np.float32)
    wB = np.zeros((64, CB), np.float32)
    wC = np.zeros((128, CC), np.float32)
    wD = np.zeros((1, CD), np.float32)

    ctw, ctb = f(ct[0]), f(ct[1])
    r1, r2, r3 = resblocks
    w11, b11 = _fold_bn(f(r1["c1w"]), f(r1["c1b"]), [f(p) for p in r1["bn1"]])
    w12, b12 = _fold_bn(f(r1["c2w"]), f(r1["c2b"]), [f(p) for p in r1["bn2"]])
    w21, b21 = _fold_bn(f(r2["c1w"]), f(r2["c1b"]), [f(p) for p in r2["bn1"]])
    w22, b22 = _fold_bn(f(r2["c2w"]), f(r2["c2b"]), [f(p) for p in r2["bn2"]])
    w31, b31 = _fold_bn(f(r3["c1w"]), f(r3["c1b"]), [f(p) for p in r3["bn1"]])
    w32, b32 = _fold_bn(f(r3["c2w"]), f(r3["c2b"]), [f(p) for p in r3["bn2"]])
    skw, skb = f(r3["skw"]), f(r3["skb"])

    W1, a1 = f(gats[0][0]), f(gats[0][1])
    W2, a2 = f(gats[1][0]), f(gats[1][1])
    W1a = W1 @ np.stack([a1[:64, 0], a1[64:, 0]], axis=1)  # [128, 2]
    W2a = W2 @ np.stack([a2[:64, 0], a2[64:, 0]], axis=1)  # [64, 2]

    wA[:, A_RB3C2:A_RB3C2 + 384] = _lhsT(w32).reshape(128, 384)
    wA[:, A_W1GAT:A_W1GAT + 64] = W1
    wA[:, A_W1A:A_W1A + 2] = W1a
    wA[:, A_W1AB:A_W1AB + 128] = np.repeat(W1a[:, 0:1], 128, axis=1)

    wB[:, B_RB1W1:B_RB1W1 + 192] = _lhsT(w11).reshape(64, 192)
    wB[:, B_RB1W2:B_RB1W2 + 192] = _lhsT(w12).reshape(64, 192)
    wB[:, B_RB2W1:B_RB2W1 + 192] = _lhsT(w21).reshape(64, 192)
    wB[:, B_RB2W2:B_RB2W2 + 192] = _lhsT(w22).reshape(64, 192)
    wB[:, B_RB3C1:B_RB3C1 + 384] = _lhsT(w31).reshape(64, 384)
    wB[:, B_SKW:B_SKW + 128] = skw[:, :, 0].T
    wB[:, B_I64:B_I64 + 64] = np.eye(64, dtype=np.float32)
    pw1, pb1, pw2, _pb2 = [f(p) for p in pool]  # pb2 shifts all logits: dropped
    wB[:, B_PW1:B_PW1 + 32] = pw1
    wB[:, B_W2GAT:B_W2GAT + 64] = W2
    wB[:, B_W2A:B_W2A + 2] = W2a
    wB[:32, B_PW2] = pw2[:, 0]
    wB[:, B_W2AB:B_W2AB + 128] = np.repeat(W2a[:, 0:1], 128, axis=1)

    wC[:64, C_B0] = ctb
    wC[:64, C_B11] = b11
    wC[:64, C_B12] = b12
    wC[:64, C_B21] = b21
    wC[:64, C_B22] = b22
    wC[:, C_B31] = b31
    wC[:, C_B32] = b32 + skb
    wC[:32, C_PB1] = pb1

    wD[0, D_CT:D_CT + 192] = _lhsT(ctw).reshape(1, 192)
    wD[0, D_ONES:D_ONES + 512] = 1.0

    xs = np.zeros((B, TP), np.float32)
    xs[:, D0:D0 + T] = f(x)[:, 0, :]
    return wA, wB, wC, wD, xs


def make_in_maps(wA, wB, wC, wD, xs):
    return [
        {"wA": wA, "wB": wB, "wC": wC, "wD": wD,
         "x": xs[i % B].reshape(1, TP).copy()}
        for i in range(N_CORES)
    ]


# ------------------------------------------------------------------ device IR

def _emit(ctx, tc, nc, wA_e, wB_e, wC_e, wD_e, x_e, out_e):
    wts = ctx.enter_context(tc.tile_pool(name="wts", bufs=1))
    acts = ctx.enter_context(tc.tile_pool(name="acts", bufs=1))
    gp = ctx.enter_context(tc.tile_pool(name="gp", bufs=2))
    ps_conv = ctx.enter_context(tc.tile_pool(name="ps_conv", bufs=2, space="PSUM"))
    ps_big = ctx.enter_context(tc.tile_pool(name="ps_big", bufs=1, space="PSUM"))
    ps_sm = ctx.enter_context(tc.tile_pool(name="ps_sm", bufs=3, space="PSUM"))

    wA = wts.tile([128, CA], F32R)
    wB = wts.tile([64, CB], F32R)
    wC = wts.tile([128, CC], F32)
    wD = wts.tile([1, CD], F32R)
    x_sb = wts.tile([1, TP], F32R)
    onescol = wts.tile([128, 1], F32)
    nc.sync.dma_start(out=x_sb[:], in_=x_e.ap())
    nc.sync.dma_start(out=wD[:], in_=wD_e.ap())
    nc.sync.dma_start(out=wC[:], in_=wC_e.ap())
    nc.sync.dma_start(out=wB[:], in_=wB_e.ap())
    nc.sync.dma_start(out=wA[:], in_=wA_e.ap())
    nc.vector.memset(onescol[:], 1.0)

    ones = wD[0:1, D_ONES:D_ONES + 512]  # [1, 512] of 1.0, f32r

    def mm(out, lhsT, rhs, start, stop):
        nc.tensor.matmul(out=out, lhsT=lhsT, rhs=rhs, start=start, stop=stop,
                         skip_group_check=True)

    def conv3(psum, h_in, w3, extra=None):
        """psum[:, t] = sum_k w3[:, k, :].T @ h_in[:, D0+t+k-1] (+ extra mms).

        h_in is a padded [C, TP] tile (zeros in cols 0:2 and 514:516), so all
        three taps write the full aligned psum [C, 0:T] (fp32r dst rule).
        """
        for k in range(3):
            mm(psum[:, :], w3[:, k, :], h_in[:, k + 1:k + 1 + T],
               k == 0, k == 2 and extra is None)
        if extra is not None:
            lhsT, rhs = extra
            mm(psum[:, :], lhsT, rhs, False, True)

    def padtile(pool_, c, **kw):
        t = pool_.tile([c, TP], F32R, **kw)
        nc.gpsimd.memset(t[:, 0:D0].bitcast(F32), 0.0)
        nc.gpsimd.memset(t[:, D0 + T:TP].bitcast(F32), 0.0)
        return t

    H = T // 2

    def epilogue(dst, psum, bias_col, relu):
        # bias+activation, split: ScalarE does the first half, VectorE the
        # second, so both halves run concurrently.
        C = psum.shape[0]
        b = wC[0:C, bias_col:bias_col + 1]
        nc.scalar.activation(out=dst[:, D0:D0 + H], in_=psum[:, 0:H],
                             func=AF.Relu if relu else AF.Identity,
                             bias=b, scale=1.0)
        if relu:
            nc.vector.tensor_scalar(out=dst[:, D0 + H:D0 + T], in0=psum[:, H:T],
                                    scalar1=b, scalar2=0.0,
                                    op0=ALU.add, op1=ALU.max)
        else:
            nc.vector.tensor_scalar_add(out=dst[:, D0 + H:D0 + T],
                                        in0=psum[:, H:T], scalar1=b)

    # ---------------- conv stack ----------------
    with nc.named_scope("convs"):
        ctw = wD[0:1, D_CT:D_CT + 192].rearrange("p (k o) -> p k o", k=3)
        h0 = padtile(acts, 64, tag="h0")
        p = ps_conv.tile([64, T], F32, tag="cps")
        conv3(p, x_sb, ctw)
        epilogue(h0, p, C_B0, relu=False)

        rb1w1 = wB[:, B_RB1W1:B_RB1W1 + 192].rearrange("p (k o) -> p k o", k=3)
        s1 = padtile(acts, 64, tag="s1")
        p = ps_conv.tile([64, T], F32, tag="cps")
        conv3(p, h0, rb1w1)
        epilogue(s1, p, C_B11, relu=True)

        rb1w2 = wB[:, B_RB1W2:B_RB1W2 + 192].rearrange("p (k o) -> p k o", k=3)
        h1 = padtile(acts, 64, tag="h1")
        p = ps_conv.tile([64, T], F32, tag="cps")
        conv3(p, s1, rb1w2, extra=(wB[:, B_I64:B_I64 + 64], h0[:, D0:D0 + T]))
        epilogue(h1, p, C_B12, relu=True)

        rb2w1 = wB[:, B_RB2W1:B_RB2W1 + 192].rearrange("p (k o) -> p k o", k=3)
        s2 = padtile(acts, 64, tag="s2")
        p = ps_conv.tile([64, T], F32, tag="cps")
        conv3(p, h1, rb2w1)
        epilogue(s2, p, C_B21, relu=True)

        rb2w2 = wB[:, B_RB2W2:B_RB2W2 + 192].rearrange("p (k o) -> p k o", k=3)
        h2 = padtile(acts, 64, tag="h2")
        p = ps_conv.tile([64, T], F32, tag="cps")
        conv3(p, s2, rb2w2, extra=(wB[:, B_I64:B_I64 + 64], h1[:, D0:D0 + T]))
        epilogue(h2, p, C_B22, relu=True)

        rb3c1 = wB[:, B_RB3C1:B_RB3C1 + 384].rearrange("p (k o) -> p k o", k=3)
        s3 = padtile(acts, 128, tag="s3")
        p = ps_conv.tile([128, T], F32, tag="cps")
        conv3(p, h2, rb3c1)
        epilogue(s3, p, C_B31, relu=True)

        rb3c2 = wA[:, A_RB3C2:A_RB3C2 + 384].rearrange("p (k o) -> p k o", k=3)
        h3 = padtile(acts, 128, tag="h3")
        p = ps_conv.tile([128, T], F32, tag="cps")
        conv3(p, s3, rb3c2, extra=(wB[:, B_SKW:B_SKW + 128], h2[:, D0:D0 + T]))
        epilogue(h3, p, C_B32, relu=True)

    # ---------------- GAT layers ----------------
    def gat(tag, hT, C, W_sb, Wa_sb, Wab_sb):
        with nc.named_scope(tag):
            # f2[j] columns, one [128, 2] matmul per j-chunk (col 1 = f2)
            f2c = gp.tile([128, 4], F32, tag="f2c")
            for jc in range(4):
                f2p = ps_sm.tile([128, 2], F32, tag="sm")
                mm(f2p[:], hT[:, jc * 128:(jc + 1) * 128], Wa_sb[:, 0:2],
                   True, True)
                nc.vector.tensor_copy(f2c[:, jc:jc + 1], f2p[:, 1:2])

            # f1 broadcast to all 128 partitions in one matmul:
            # f1b[j, n] = sum_c Wab[c, j] h[c, n] with Wab[c, j] = (W a1)[c]
            f1b = ps_big.tile([128, T], F32, tag="f1b")
            mm(f1b[:], Wab_sb[:], hT[:, :], True, True)

            # Wh chunks [n, 64] with an appended ones column (-> Z row)
            wh = []
            for jc in range(4):
                wp = ps_sm.tile([128, 64], F32, tag="sm")
                mm(wp[:], hT[:, jc * 128:(jc + 1) * 128], W_sb, True, True)
                ws = gp.tile([128, 65], F32R, tag="whsb")
                nc.vector.tensor_copy(ws[:, 0:64], wp[:])
                nc.vector.tensor_copy(ws[:, 64:65], onescol[:])
                wh.append(ws)

            # scores + aggregation, pipelined per chunk
            o_ps = ps_big.tile([65, T], F32, tag="ops")
            for jc in range(4):
                lre = gp.tile([128, T], F32R, tag="lre")
                nc.scalar.activation(out=lre[:], in_=f1b[:], func=AF.Prelu,
                                     alpha=ALPHA, bias=f2c[:, jc:jc + 1],
                                     scale=1.0)
                expE = gp.tile([128, T], F32R, tag="expE")
                nc.scalar.activation(out=expE[:], in_=lre[:], func=AF.Exp)
                mm(o_ps[:], wh[jc][:], expE[:], jc == 0, jc == 3)

            # h' = elu(O / Z); 1/Z via matmul-broadcast + fast reciprocal
            zrow = gp.tile([1, T], F32R, tag="zrow")
            nc.scalar.activation(out=zrow[:], in_=o_ps[64:65, :],
                                 func=AF.Identity)
            zb = ps_sm.tile([64, T], F32, tag="sm")
            mm(zb[:], ones[:, 0:64], zrow[:], True, True)
            rzb = gp.tile([64, T], F32, tag="rzb")
            nc.vector.reciprocal_approx_fast(out=rzb[:], in_=zb[:])

            q = gp.tile([64, T], F32, tag="q")
            nc.vector.tensor_tensor(out=q[:], in0=o_ps[0:64, :], in1=rzb[:],
                                    op=ALU.mult)
            e1 = gp.tile([64, T], F32, tag="e1")
            nc.scalar.activation(out=e1[:], in_=q[:], func=AF.Exp)
            m = gp.tile([64, T], F32, tag="m")
            nc.vector.tensor_scalar(out=m[:], in0=e1[:], scalar1=1.0,
                                    scalar2=0.0, op0=ALU.subtract, op1=ALU.min)
            hn = gp.tile([64, T], F32R, tag="hn")
            nc.vector.scalar_tensor_tensor(out=hn[:], in0=q[:], scalar=0.0,
                                           in1=m[:], op0=ALU.max, op1=ALU.add)
            return hn

    h1p = gat("gat1", h3[:, D0:D0 + T], 128, wA[:, A_W1GAT:A_W1GAT + 64],
              wA[:, A_W1A:A_W1A + 2], wA[:, A_W1AB:A_W1AB + 128])
    h2p = gat("gat2", h1p, 64, wB[:, B_W2GAT:B_W2GAT + 64],
              wB[:, B_W2A:B_W2A + 2], wB[:, B_W2AB:B_W2AB + 128])

    # ---------------- attention pool ----------------
    with nc.named_scope("pool"):
        t1p = ps_sm.tile([32, T], F32, tag="sm")
        mm(t1p[:], wB[:, B_PW1:B_PW1 + 32], h2p[:], True, True)
        t1 = gp.tile([32, T], F32R, tag="t1")
        nc.scalar.activation(out=t1[:], in_=t1p[:], func=AF.Relu,
                             bias=wC[0:32, C_PB1:C_PB1 + 1], scale=1.0)

        lp = ps_sm.tile([1, T], F32, tag="sm")
        mm(lp[:], wB[0:32, B_PW2:B_PW2 + 1], t1[:], True, True)
        expl = gp.tile([1, T], F32R, tag="expl")
        z2 = gp.tile([1, 1], F32, tag="z2")
        nc.scalar.activation(out=expl[:], in_=lp[:], func=AF.Exp,
                             accum_out=z2[:])

        eb = ps_sm.tile([64, T], F32, tag="sm")
        mm(eb[:], ones[:, 0:64], expl[:], True, True)
        w_sb = gp.tile([64, T], F32, tag="w_sb")
        oun = gp.tile([64, 1], F32, tag="oun")
        nc.vector.tensor_tensor(out=w_sb[:], in0=h2p[:].bitcast(F32),
                                in1=eb[:], op=ALU.mult)
        nc.vector.reduce_sum(out=oun[:], in_=w_sb[:], axis=mybir.AxisListType.X)
        nc.sync.dma_start(out=out_e.ap()[0:64, :], in_=oun[:])
        nc.sync.dma_start(out=out_e.ap()[64:65, :], in_=z2[:])


_GRAPH = None


def _build():
    global _GRAPH
    if _GRAPH is not None:
        return _GRAPH
    nc = bacc.Bacc("TRN2", target_bir_lowering=False, debug=False,
                   num_devices=N_CORES)
    wA_e = nc.dram_tensor("wA", [128, CA], F32R, kind="ExternalInput")
    wB_e = nc.dram_tensor("wB", [64, CB], F32R, kind="ExternalInput")
    wC_e = nc.dram_tensor("wC", [128, CC], F32, kind="ExternalInput")
    wD_e = nc.dram_tensor("wD", [1, CD], F32R, kind="ExternalInput")
    x_e = nc.dram_tensor("x", [1, TP], F32R, kind="ExternalInput")
    out_e = nc.dram_tensor("out", [65, 1], F32, kind="ExternalOutput")
    from contextlib import ExitStack
    with tile.TileContext(nc) as tc:
        with ExitStack() as ctx:
            _emit(ctx, tc, nc, wA_e, wB_e, wC_e, wD_e, x_e, out_e)
    nc.compile()
    _GRAPH = nc
    return nc


def kernel(x, ct, resblocks, gats, pool, _want_results=False, **_ignored):
    wA, wB, wC, wD, xs = _prep(x, ct, resblocks, gats, pool)
    nc = _build()
    in_maps = make_in_maps(wA, wB, wC, wD, xs)
    res = run_bass_kernel_spmd(nc, in_maps, core_ids=list(range(N_CORES)))
    out = np.empty((B, 64), np.float32)
    for b in range(B):
        v = res.results[b]["out"][:, 0]
        out[b] = v[0:64] / v[64]
    if _want_results:
        return out, res
    return out
